# revision 33
# baseline (speedup 1.0000x reference)
"""Multi-head attention on 8 TRN2 NeuronCores.

Sharding: 4-way data-parallel over batch x 2-way tensor-parallel over heads.
Core c handles batch (c // 2) and heads [8*(c%2), 8*(c%2)+8).

Per-core kernel (feature-major / transposed layouts throughout):
  xT   [1024, 2048]  (bf16, d-major)           -> SBUF
  Q^T, K^T = Wq/Wk slices @ xT + bias           [512, 2048] (e-major, bf16)
             (1/sqrt(dk) folded into Wq, bq on host)
  V    = x @ Wv^T slice + bias, token-major     [2048, 8 heads, 64+1] fp8e4m3
         (65th column = ones -> softmax denominator comes free in ctx matmul)
  S^T[k,q] = K^T.T @ Q^T per head, TWO heads packed in the 128-row PE array
             via tile_position row groups (FD=512, f32 psum).
  P = exp(S^T): ONE ACTIVATE per key-tile covers both heads = FD 1024
      (f32 psum in, fp8e4m3 out; scores are small so no max-subtraction;
      bf16 psum matmul output would enable FD-2048 but is TRN3-only).
  ctx^T[d,q] (+denom row) = V.T @ P via fp8 DoubleRow matmuls (K=256 per
      instruction), FD=512 (f32 psum accumulation cap), 8 kt-pairs.
  normalize: copy ctx psum to SBUF, DVE reciprocal_approx_fast on the
      denom row, broadcast across 64 partitions via a DRAM roundtrip on
      the otherwise-idle gpsimd/sync DMA queues (zero TensorE cost), DVE
      multiply into ctxT (bf16).  The last block uses a PE ones-matmul
      broadcast instead (PE is idle in the tail, roundtrip latency isn't).
  outT_partial[e,t] = Wo^T slice.T @ ctx^T      [1024, 2048] bf16 -> DRAM

Host: out[b] = (outT_core(2b) + outT_core(2b+1)).T + bo.

Scheduling (engines execute their streams IN ORDER; ScalarE exp is the
~277us bottleneck, TensorE wall is a close ~295us second):
  - Blocks are (head-pair, 512-query) score/exp passes: hp0, hp1 phases
    run their four query tiles back to back; hp2/hp3 interleave per query
    tile so O-projection(qslice) unlocks mid-phase instead of at the end.
  - ctx matmuls + normalize of block i run (mostly) during block i+1 via
    a global work queue drained at a measured-cost budget per kt slot
    (ctx pairs 0-3 queued ahead of 4-7 to give x halves 2-3 DMA headroom);
    V/O units flow through the same queue; K/Q units have hard pre-slot
    deadlines emitted AFTER each slot's ACTIVATE so they never delay exp.
    The es pool holds 3 blocks of exp tiles so ACT(b) only waits on
    ctx(b-3); force-drain keeps that invariant deadlock-free.
  - Startup: critical DMAs (x half0, Wq/Wk hp0 chunks) are issued first
    on the 3 DMA queues; wv + x halves 1-3 ride the second wave.
  - The budget pacing constants (COST_*, ACT_US_PER_SLOT, budget=4.0)
    are empirically load-bearing: exec is sensitive at the +/-5us level
    to the initial budget (0.0 collapsed the early schedule to 408us).
"""

import numpy as np
import ml_dtypes
from contextlib import ExitStack

import concourse.bass as bass
import concourse.bacc as bacc
import concourse.mybir as mybir
import concourse.tile as tile
from concourse.bass_utils import run_bass_kernel_spmd


D = 1024          # d_model
HEADS = 16
DK = 64           # head dim
B = 4             # batch
S = 2048          # sequence length
TP = 2            # tensor-parallel ways (over heads)
DP = 4            # data-parallel ways (over batch)
N_CORES = 8
EL = D // TP      # 512 local projection dims
HL = HEADS // TP  # 8 local heads
HP = EL // 128    # 4 head-pairs per core
T = S             # tokens per core (one batch)
KT = D // 128     # 8 contraction tiles for projections
TT = T // 128     # 16 token tiles
NK = T // 128     # 16 key tiles per block
NP = NK // 2      # 8 key-tile PAIRS (fp8 DoubleRow contracts 2 kt at once)

F32 = mybir.dt.float32
BF16 = mybir.dt.bfloat16
F8 = mybir.dt.float8e4
AF = mybir.ActivationFunctionType
ALU = mybir.AluOpType
DR = mybir.MatmulPerfMode.DoubleRow

VP = 128
VD = 64            # first V dim column in the fp8 V tile

# Blocks: (hp, q0) 512-query score/exp passes.  hp0 and hp1 run their four
# query tiles back to back; hp2/hp3 interleave per query tile so O(qslice)
# unlocks right after the (hp2,qs)+(hp3,qs) pair instead of all at the end.
BLOCKS = ([(0, qt * 512) for qt in range(4)]
          + [(1, qt * 512) for qt in range(4)]
          + [(h, qt * 512) for qt in range(4) for h in (2, 3)])

# Rough PE-side cost (us) per work item kind, used only for budget pacing.
COST_V = 1.75
COST_QK = 1.75
COST_CTX = 0.48
COST_NORM = 0.75
COST_O = 1.05
ACT_US_PER_SLOT = 1.04          # (1024+222)/1.2 per ACTIVATE
ST_US = 0.45                    # S^T pair wall per kt


def build_program() -> bass.Bass:
    nc = bacc.Bacc("TRN2", debug=False)

    # Inputs, pre-swizzled on the host into partition-major layouts so each
    # DMA is 128 large descriptors.  wq/wk are head-pair-major so the hp0
    # chunk can be DMA'd first (critical path to the first exp).
    xpm = nc.dram_tensor("xpm", [128, 4, KT, 512], BF16, kind="ExternalInput").ap()
    wqpm = nc.dram_tensor("wqpm", [128, HP, KT, 128], BF16, kind="ExternalInput").ap()
    wkpm = nc.dram_tensor("wkpm", [128, HP, KT, 128], BF16, kind="ExternalInput").ap()
    wvpm = nc.dram_tensor("wvpm", [128, KT, EL], BF16, kind="ExternalInput").ap()
    wopm = nc.dram_tensor("wopm", [128, HP, D], BF16, kind="ExternalInput").ap()
    bq = nc.dram_tensor("bq", [EL], F32, kind="ExternalInput").ap()
    bqf = nc.dram_tensor("bqf", [1, EL], BF16, kind="ExternalInput").ap()
    bkf = nc.dram_tensor("bkf", [1, EL], BF16, kind="ExternalInput").ap()
    bk = nc.dram_tensor("bk", [EL], F32, kind="ExternalInput").ap()
    bv = nc.dram_tensor("bv", [128, HL, DK], F32, kind="ExternalInput").ap()
    outT = nc.dram_tensor("outT", [D, T], BF16, kind="ExternalOutput").ap()

    with ExitStack() as ctx:
        tc = ctx.enter_context(tile.TileContext(nc))
        const = ctx.enter_context(tc.tile_pool(name="const", bufs=1))
        xw = ctx.enter_context(tc.tile_pool(name="xw", bufs=1))
        qkv = ctx.enter_context(tc.tile_pool(name="qkv", bufs=1))
        expp = ctx.enter_context(tc.tile_pool(name="expp", bufs=24))
        stage = ctx.enter_context(tc.tile_pool(name="stage", bufs=3))
        psp = ctx.enter_context(tc.tile_pool(name="psp", bufs=2, space="PSUM"))
        ctxp = ctx.enter_context(tc.tile_pool(name="ctxp", bufs=2, space="PSUM"))
        fillp = ctx.enter_context(tc.tile_pool(name="fillp", bufs=2, space="PSUM"))
        drp = ctx.enter_context(tc.tile_pool(name="drp", bufs=3, space="DRAM"))

        # ---------------- loads (critical first, in waves) ----------------
        xt_sb = xw.tile([128, 4, KT, 512], BF16)     # [p, half, kt, t%512]
        wq_sb = xw.tile([128, HP, KT, 128], BF16)
        wk_sb = xw.tile([128, HP, KT, 128], BF16)
        wv_sb = xw.tile([128, KT, EL], BF16)
        wo_sb = xw.tile([128, HP, D], BF16)

        bq_sb = const.tile([128, HP], F32)
        bqf_sb = const.tile([1, EL], BF16)
        bkf_sb = const.tile([1, EL], BF16)
        bk_sb = const.tile([128, HP], F32)
        bv_sb = const.tile([128, HL, DK], F32)

        # Wave 1: exactly what the first S^T/exp needs, one tensor per queue
        # (only sync/scalar/gpsimd can issue DMAs).
        nc.scalar.dma_start(out=bqf_sb, in_=bqf)
        nc.gpsimd.dma_start(out=bkf_sb, in_=bkf)
        nc.sync.dma_start(out=xt_sb[:, 0, :, :], in_=xpm[:, 0, :, :])
        nc.scalar.dma_start(out=wq_sb[:, 0, :, :], in_=wqpm[:, 0, :, :])
        nc.gpsimd.dma_start(out=wk_sb[:, 0, :, :], in_=wkpm[:, 0, :, :])
        # Wave 2: wv + the remaining x halves — every V tile is consumed by
        # ctx(B0) during B1 (t ~= 40-60us), so x2/x3 are nearly as urgent as
        # wv; the big weight remainders can wait until ~75us (hp1 staging).
        nc.sync.dma_start(out=wv_sb, in_=wvpm)
        nc.gpsimd.dma_start(out=bv_sb, in_=bv)
        nc.scalar.dma_start(out=xt_sb[:, 2, :, :], in_=xpm[:, 2, :, :])
        nc.gpsimd.dma_start(out=xt_sb[:, 3, :, :], in_=xpm[:, 3, :, :])
        nc.sync.dma_start(out=xt_sb[:, 1, :, :], in_=xpm[:, 1, :, :])
        # Wave 3: weight remainders (first needed by hp1 staging ~75us in)
        # and Wo (needed ~180us in).
        nc.scalar.dma_start(out=bq_sb, in_=bq.rearrange("(a p) -> p a", p=128))
        nc.scalar.dma_start(out=wq_sb[:, 1:, :, :], in_=wqpm[:, 1:, :, :])
        nc.gpsimd.dma_start(out=bk_sb, in_=bk.rearrange("(a p) -> p a", p=128))
        nc.gpsimd.dma_start(out=wk_sb[:, 1:, :, :], in_=wkpm[:, 1:, :, :])
        nc.sync.dma_start(out=wo_sb, in_=wopm)

        # PE warmup against the idle-clock p-state while DMAs stream.
        wu_sb = const.tile([128, 512], BF16)
        nc.vector.memset(wu_sb, 1.0)
        ones_f = const.tile([1, 128], F32)
        nc.vector.memset(ones_f, 1.0)
        wu_ps = psp.tile([128, 2, 512], F32, tag="ps", name="warm")
        for j in range(6):
            nc.tensor.matmul(wu_ps[:, j % 2, :],
                             lhsT=wu_sb[:, 0:128], rhs=wu_sb,
                             start=True, stop=True)

        # ---------------- persistent SBUF state ----------------
        qt_sb = qkv.tile([128, HP, T], BF16)   # [p(=e within hp), hp, t]
        kt_sb = qkv.tile([128, HP, T], BF16)
        v_sb = qkv.tile([128, TT // 2, HL, 2, VP], F8)
        nc.vector.memset(v_sb[:, :, :, :, 0:1], 1.0)
        ctxT_sb = qkv.tile([128, HP, T], BF16)  # [p(=d within hp), hp, q]
        outT_r = outT.rearrange("(E p) t -> p E t", p=128)  # [128, 8, 2048]

        # ---------------- unit builders ----------------
        def qk_unit(which, hp, half, act_copy=False):
            w_sb, b_sb, dst = ((wq_sb, bq_sb, qt_sb) if which == 0
                               else (wk_sb, bk_sb, kt_sb))
            bf_sb = bqf_sb if which == 0 else bkf_sb
            t0 = half * 512
            fp = fillp.tile([128, 512], F32, tag="fill", name="fp")
            for kt in range(KT):
                nc.tensor.matmul(
                    fp,
                    lhsT=w_sb[:, hp, kt, :],
                    rhs=xt_sb[:, half, kt, :],
                    start=(kt == 0), stop=(kt == KT - 1 and not act_copy))
            if act_copy:
                # bias lands in PSUM via a ones-row matmul; ScalarE does the
                # psum->sbuf move (keeps DVE latency off the startup path)
                nc.tensor.matmul(
                    fp, lhsT=bf_sb[0:1, hp * 128:(hp + 1) * 128],
                    rhs=wu_sb[0:1, :], start=False, stop=True)
                nc.scalar.copy(out=dst[:, hp, t0:t0 + 512], in_=fp)
            else:
                nc.vector.tensor_scalar_add(
                    out=dst[:, hp, t0:t0 + 512], in0=fp, scalar1=b_sb[:, hp:hp + 1])

        def v_unit(tt):
            fp = fillp.tile([128, 512], F32, tag="fill", name="fpv")
            for kt in range(KT):
                nc.tensor.matmul(
                    fp,
                    lhsT=xt_sb[:, tt // 4, kt, (tt % 4) * 128:(tt % 4 + 1) * 128],
                    rhs=wv_sb[:, kt, :],
                    start=(kt == 0), stop=(kt == KT - 1))
            nc.vector.tensor_tensor(
                out=v_sb[:, tt // 2, :, tt % 2, VD:VD + DK],
                in0=fp.rearrange("p (h d) -> p h d", h=HL),
                in1=bv_sb, op=ALU.add)

        def oproj_unit(q0, et, W=512, queue=None, scalar_evac=False):
            fp = fillp.tile([128, 512], F32, tag="fill", name="fpo")
            for hp in range(HP):
                nc.tensor.matmul(
                    fp[:, 0:W],
                    lhsT=wo_sb[:, hp, et * 128:(et + 1) * 128],
                    rhs=ctxT_sb[:, hp, q0:q0 + W],
                    start=(hp == 0), stop=(hp == HP - 1))
            ot = stage.tile([128, 512], BF16, tag="ot", name="ot")
            if scalar_evac:
                # tail path: the exp stream has drained, ScalarE is idle
                nc.scalar.copy(out=ot[:, 0:W], in_=fp[:, 0:W])
            else:
                nc.vector.tensor_copy(ot[:, 0:W], fp[:, 0:W])
            eng = queue if queue is not None else nc.sync
            eng.dma_start(out=outT_r[:, et, q0:q0 + W], in_=ot[:, 0:W])

        # ---------------- block state ----------------
        # es tiles per (block, pair); ctx psum per (block, head)
        es_tiles = {}          # (bi, pair) -> es tile
        ctx_ps = {}            # (bi, head) -> psum tile

        def ctx_dr(bi, hh, p):
            """One DoubleRow ctx matmul: block bi, head-in-pair hh, pair p."""
            hp, q0 = BLOCKS[bi]
            h = 2 * hp + hh
            key = (bi, hh)
            if key not in ctx_ps:
                ctx_ps[key] = ctxp.tile([128, 512], F32, tag="ctx", name="ctx")
            es = es_tiles[(bi, p)]
            nc.tensor.matmul(
                ctx_ps[key],
                lhsT=v_sb[:, p, h, :, :],
                rhs=es[:, hh, :, :],
                start=(p == 0), stop=(p == NP - 1), perf_mode=DR)

        def _bcast_ap(ap, parts):
            """Prepend a step-0 partition dim (DRAM-side broadcast)."""
            return bass.AP(tensor=ap.tensor, offset=ap.offset,
                           ap=[[0, parts]] + list(ap.ap))

        def norm(bi, hh):
            """Normalize one ctx head.

            Default path (mid-kernel): copy ctx psum to SBUF (frees the
            bank), reciprocal of the denom row, broadcast the reciprocal
            across 64 partitions via a DRAM roundtrip on the (otherwise
            idle) gpsimd+sync DMA queues — zero TensorE cost — then one
            DVE multiply into ctxT (bf16).

            Tail path (last two blocks): PE ones-matmul broadcast instead
            of the DMA roundtrip — the PE is idle there and the roundtrip
            latency would sit on the critical path.
            """
            hp, q0 = BLOCKS[bi]
            cps = ctx_ps.pop((bi, hh))
            rec = stage.tile([1, 512], F32, tag="rec", name="rec")
            r0 = hh * DK
            if bi >= 15:
                nc.vector.reciprocal_approx_fast(out=rec, in_=cps[0:1, :])
                bc = fillp.tile([128, 512], F32, tag="fill", name="bc")
                nc.tensor.matmul(bc, lhsT=ones_f, rhs=rec,
                                 start=True, stop=True)
                bcs = stage.tile([VP, 512], F32, tag="bcs", name="bcs")
                nc.vector.tensor_copy(bcs[VD:VD + DK, :], bc[VD:VD + DK, :])
                nc.vector.tensor_tensor(
                    out=ctxT_sb[r0:r0 + DK, hp, q0:q0 + 512],
                    in0=cps[VD:VD + DK, :], in1=bcs[VD:VD + DK, :],
                    op=ALU.mult)
            else:
                cc = stage.tile([VP, 512], F32, tag="cc", name="cc")
                nc.vector.tensor_copy(cc, cps)
                nc.vector.reciprocal_approx_fast(out=rec, in_=cc[0:1, :])
                rec_dr = drp.tile([1, 512], F32, tag="rdr", name="rdr")
                eng1, eng2 = ((nc.gpsimd, nc.sync) if hh == 0
                              else (nc.sync, nc.gpsimd))
                eng1.dma_start(out=rec_dr, in_=rec)
                bcs = stage.tile([VP, 512], F32, tag="bcs", name="bcs")
                eng2.dma_start(out=bcs[VD:VD + DK, :],
                               in_=_bcast_ap(rec_dr[0, :], DK))
                nc.vector.tensor_tensor(
                    out=ctxT_sb[r0:r0 + DK, hp, q0:q0 + 512],
                    in0=cc[VD:VD + DK, :], in1=bcs[VD:VD + DK, :],
                    op=ALU.mult)
            if hh == 1:   # last subseq of the block: drop the es refs
                for p in range(NP):
                    es_tiles.pop((bi, p), None)

        # ---------------- global work queue ----------------
        # Each item: (cost_us, earliest (block,kt) or None, thunk).  Queue
        # order IS the PE stream order for queued work; correctness comes
        # from tile semaphores, order only shapes timing — EXCEPT that
        # (a) an item may reference tiles that only exist after `earliest`,
        # (b) ctx/norm of block b must be fully emitted before block b+2
        #     starts (pool-rotation waits would deadlock otherwise) —
        #     enforced via force_map.
        queue = []
        force_map = {}

        def q_add(cost, earliest, fn, *a, **kw):
            queue.append((cost, earliest, lambda: fn(*a, **kw)))

        def q_ctx_block(bi):
            # pairs 0-3 for both heads first, then pairs 4-7: the late V
            # tiles (x halves 2-3) get two extra blocks of DMA headroom
            for p_grp in (range(0, 4), range(4, NP)):
                for hh in range(2):
                    for p in p_grp:
                        q_add(COST_CTX, (bi, 2 * p + 1), ctx_dr, bi, hh, p)
            for hh in range(2):
                # the DMA-roundtrip norm costs the PE nothing; the tail
                # PE-broadcast variant costs ~one bcast matmul
                q_add(0.2 if bi < 15 else COST_NORM, (bi, 15), norm, bi, hh)
            # es pool holds 24 tiles = 3 blocks; ACT of block b waits on
            # ctx readers of block b-3, so those must be emitted by then.
            force_map[bi + 3] = len(queue)

        # V0..V7 flow during B0 (x half0/half1 + wv gated); V8..V15 precede
        # the ctx(B0) pairs that consume them (draining during B1).
        for tt in range(16):
            q_add(COST_V, None, v_unit, tt)
        for bi in range(9):
            q_ctx_block(bi)
        # O(qslice) unlocks after the (hp2,qs)/(hp3,qs) block pair's norms
        # (hp0/hp1 norms are long done): blocks 8,9 -> q0; 10,11 -> 512; ...
        q_ctx_block(9)
        for et in range(8):
            q_add(COST_O, None, oproj_unit, 0, et)
        q_ctx_block(10)
        q_ctx_block(11)
        for et in range(8):
            q_add(COST_O, None, oproj_unit, 512, et)
        q_ctx_block(12)
        q_ctx_block(13)
        for et in range(8):
            q_add(COST_O, None, oproj_unit, 1024, et)
        q_ctx_block(14)
        q_ctx_block(15)
        for et in range(8):
            q_add(COST_O, None, oproj_unit, 1536, et,
                  queue=(nc.scalar if et % 2 else nc.sync))

        # Hard pre-slot deadlines: (block_idx, kt) -> list of thunks.
        # Within each hp phase: K(hp, h) before S^T kt=4h of the phase's
        # first block; Q(hp, qt) before block (hp, qt); next hp's K/Q staged
        # late in the previous phase.
        pre = {}

        def pre_add(bi, kt, fn, *a, **kw):
            pre.setdefault((bi, kt), []).append(lambda: fn(*a, **kw))

        pre_add(0, 3, qk_unit, 1, 0, 1)      # K(hp0,h1) by B0 kt4
        pre_add(0, 7, qk_unit, 1, 0, 2)
        pre_add(0, 11, qk_unit, 1, 0, 3)
        pre_add(0, 13, qk_unit, 0, 0, 1)     # Q(hp0,qt1) for B1
        pre_add(1, 8, qk_unit, 0, 0, 2)
        pre_add(2, 8, qk_unit, 0, 0, 3)
        pre_add(3, 2, qk_unit, 0, 1, 0)      # hp1 staging late in hp0 phase
        pre_add(3, 5, qk_unit, 1, 1, 0)
        pre_add(3, 9, qk_unit, 1, 1, 1)
        pre_add(3, 12, qk_unit, 1, 1, 2)
        pre_add(4, 8, qk_unit, 1, 1, 3)      # by B4 kt12
        pre_add(4, 12, qk_unit, 0, 1, 1)
        pre_add(5, 8, qk_unit, 0, 1, 2)
        pre_add(6, 8, qk_unit, 0, 1, 3)
        pre_add(7, 2, qk_unit, 0, 2, 0)      # hp2 staging late in hp1 phase
        pre_add(7, 5, qk_unit, 1, 2, 0)
        pre_add(7, 9, qk_unit, 1, 2, 1)
        pre_add(7, 12, qk_unit, 1, 2, 2)
        # B8=(2,0): stage hp3's first units (B9=(3,0) follows immediately)
        pre_add(8, 2, qk_unit, 0, 3, 0)
        pre_add(8, 6, qk_unit, 1, 2, 3)      # K(hp2,h3) by B8 kt12
        pre_add(8, 9, qk_unit, 1, 3, 0)
        pre_add(8, 13, qk_unit, 1, 3, 1)     # by B9 kt4
        pre_add(9, 2, qk_unit, 1, 3, 2)      # by B9 kt8
        pre_add(9, 6, qk_unit, 1, 3, 3)      # by B9 kt12
        pre_add(9, 10, qk_unit, 0, 2, 1)     # Q(hp2,qt1) for B10
        pre_add(10, 10, qk_unit, 0, 3, 1)    # Q(hp3,qt1) for B11
        pre_add(11, 10, qk_unit, 0, 2, 2)
        pre_add(12, 10, qk_unit, 0, 3, 2)
        pre_add(13, 10, qk_unit, 0, 2, 3)
        pre_add(14, 10, qk_unit, 0, 3, 3)

        # ---------------- emission ----------------
        # Upfront: only what the very first S^T needs.
        qk_unit(0, 0, 0, act_copy=True)   # Q(hp0, qt0)
        qk_unit(1, 0, 0, act_copy=True)   # K(hp0, h0)

        qi = 0                 # queue drain pointer
        spent = 0.0            # us of queued work emitted
        budget = 4.0           # us of queue slack (ACT idles a bit at start)

        def eligible(bi, kt):
            if qi >= len(queue):
                return False
            _, earliest, _ = queue[qi]
            return earliest is None or earliest <= (bi, kt)

        for bi, (hp, q0) in enumerate(BLOCKS):
            for kt in range(NK):
                if kt == 1:
                    # force-drain: ctx/norm of block bi-3 must be in the PE
                    # stream before this block's psum/es-pool rotations wait
                    # on them.  Done after slot 0 so the block's first S^T
                    # (and its ACTIVATE) aren't delayed by leftovers.
                    while qi < force_map.get(bi, 0):
                        cost, _, fn = queue[qi]
                        fn()
                        spent += cost
                        qi += 1
                # S^T pair for this kt (two heads in row groups 0-63/64-127)
                ps = psp.tile([128, 2, 512], F32, tag="ps", name="pss")
                nc.tensor.matmul(
                    ps[:, 0, :],
                    lhsT=kt_sb[0:64, hp, kt * 128:(kt + 1) * 128],
                    rhs=qt_sb[0:64, hp, q0:q0 + 512],
                    start=True, stop=True)
                nc.tensor.matmul(
                    ps[:, 1, :],
                    lhsT=kt_sb[64:128, hp, kt * 128:(kt + 1) * 128],
                    rhs=qt_sb[64:128, hp, q0:q0 + 512],
                    start=True, stop=True, tile_position=(64, 0))
                if kt % 2 == 0:
                    es = expp.tile([128, 2, 2, 512], F8, tag="es", name="es")
                    es_tiles[(bi, kt // 2)] = es
                else:
                    es = es_tiles[(bi, kt // 2)]
                nc.scalar.activation(
                    out=es[:, :, kt % 2, :],
                    in_=ps,
                    func=AF.Exp)
                # pre units AFTER the slot's S^T/ACT so they never delay the
                # exp stream; a pre item at (bi, kt) lands before S^T(kt+1)
                for fn in pre.pop((bi, kt), []):
                    fn()
                    budget -= COST_QK   # pre items eat the same PE budget
                budget += ACT_US_PER_SLOT - ST_US
                while (eligible(bi, kt)
                       and spent + queue[qi][0] * 0.5 <= budget):
                    cost, _, fn = queue[qi]
                    fn()
                    spent += cost
                    qi += 1
        # Drain whatever is left (tail).
        while qi < len(queue):
            queue[qi][2]()
            qi += 1

    nc.compile()
    return nc


_PROG = None


def _get_prog() -> bass.Bass:
    global _PROG
    if _PROG is None:
        _PROG = build_program()
    return _PROG


def make_in_maps(x, Wq, bq, Wk, bk, Wv, bv, Wo, bo):
    """Build the 8 per-core input dicts from the full (unsharded) inputs."""
    bf = ml_dtypes.bfloat16
    x = np.asarray(x, np.float32)
    scale = np.float32(1.0 / np.sqrt(DK))
    WqT = np.asarray(Wq, np.float32).T * scale   # [d, e], scores scale folded in
    WkT = np.asarray(Wk, np.float32).T
    WvT = np.asarray(Wv, np.float32).T
    WoT = np.asarray(Wo, np.float32).T           # [d_in, e_out]; rows = ctx dims
    bq = np.asarray(bq, np.float32) * scale
    bk = np.asarray(bk, np.float32)
    bv = np.asarray(bv, np.float32)

    def pm_hp(wT):
        # [D, E] (d-major) -> [128(p), HP, KT, 128]: per-(p, hp) contiguous
        return np.ascontiguousarray(
            wT.reshape(KT, 128, HP, 128).transpose(1, 2, 0, 3)).astype(bf)

    def pm(wT):
        # [D, E] -> partition-major [128, D//128, E]
        return np.ascontiguousarray(
            wT.reshape(KT, 128, wT.shape[1]).transpose(1, 0, 2)).astype(bf)

    xpm_b = [np.ascontiguousarray(
        x[b_].T.reshape(KT, 128, 4, 512).transpose(1, 2, 0, 3)).astype(bf)
        for b_ in range(B)]
    in_maps = []
    for c in range(N_CORES):
        b_idx, h2 = divmod(c, TP)
        sl = slice(h2 * EL, (h2 + 1) * EL)
        wo = WoT[sl, :]
        bv_loc = np.broadcast_to(
            bv[sl].reshape(1, HL, DK), (128, HL, DK))
        in_maps.append({
            "xpm": xpm_b[b_idx],
            "wqpm": pm_hp(np.ascontiguousarray(WqT[:, sl])),
            "wkpm": pm_hp(np.ascontiguousarray(WkT[:, sl])),
            "wvpm": pm(np.ascontiguousarray(WvT[:, sl])),
            "wopm": np.ascontiguousarray(
                wo.reshape(HP, 128, D).transpose(1, 0, 2)).astype(bf),
            "bq": np.ascontiguousarray(bq[sl]),
            "bqf": np.ascontiguousarray(bq[sl]).reshape(1, EL).astype(bf),
            "bkf": np.ascontiguousarray(bk[sl]).reshape(1, EL).astype(bf),
            "bk": np.ascontiguousarray(bk[sl]),
            "bv": np.ascontiguousarray(bv_loc, dtype=np.float32),
        })
    return in_maps


def assemble_output(results, bo):
    """Sum TP partials, transpose back to [B, S, D], add output bias."""
    bo32 = np.asarray(bo, np.float32)
    out = np.empty((B, S, D), np.float32)
    for b_idx in range(B):
        acc = (results[TP * b_idx]["outT"].astype(np.float32)
               + results[TP * b_idx + 1]["outT"].astype(np.float32))
        out[b_idx] = acc.T + bo32
    return out


def kernel(x, Wq, bq, Wk, bk, Wv, bv, Wo, bo):
    nc = _get_prog()
    in_maps = make_in_maps(x, Wq, bq, Wk, bk, Wv, bv, Wo, bo)
    res = run_bass_kernel_spmd(nc, in_maps, core_ids=list(range(N_CORES)))
    return assemble_output(res.results, bo)
